# revision 1
# baseline (speedup 1.0000x reference)
"""Trainium2 Bass kernel for the 4-layer spiking-MLP critic (T=16 IF/LIF recurrence).

Strategy
- Data-parallel over 8 NeuronCores: batch 4096 -> 512 per core; weights replicated.
- Everything runs transposed (feature dim on partitions, batch on the free dim),
  so no on-device transposes are needed anywhere.
- x @ W1.T + b1 is time-invariant: computed once into SBUF, reused all 16 steps.
- Weights and spikes are fp16; full fp32 accuracy is recovered with a hi/lo
  split: W ~= Whi + 2^-11 * Wlo (both fp16). Spikes are 0/1 (exact in fp16), so
  each layer is two fp16 matmul groups; the lo PSUM is folded in with a single
  fused scalar_tensor_tensor op ((lo * 2^-11) + hi).
- Layer-4 (non-spiking LIF, tau=2) is algebraically unrolled:
      v4_T = 2^-16 * sum_t 2^t * (s3_t @ W4.T) + (1 - 2^-16) * b4
  The weighted sum accumulates directly in a persistent PSUM bank across all 16
  steps by scaling the spike tensor with 2^t (exact in fp16), eliminating all
  per-step layer-4 elementwise work and state.
- IF membrane states carry their bias folded in (vb = v + b), saving one
  elementwise op per layer per step.
"""

import sys

sys.path.insert(0, "/opt/trn_rl_repo")

import numpy as np

P = 128
D, H, AOUT = 512, 1024, 64
N = 512           # batch per core
T = 16
KD, KH = D // P, H // P
CLO = float(2.0 ** -11)
NCORES = 8

_CACHE = {}


def _build():
    from contextlib import ExitStack
    from concourse import bacc, mybir, tile

    f32 = mybir.dt.float32
    f16 = mybir.dt.float16
    A = mybir.AluOpType
    IDENT = mybir.ActivationFunctionType.Identity

    nc = bacc.Bacc("TRN2", target_bir_lowering=False, debug=False)

    din = {}
    for name, shape, dt_ in [
        ("xh", [D, N], f16), ("xl", [D, N], f16),
        ("w1h", [D, H], f16), ("w1l", [D, H], f16),
        ("w2h", [H, H], f16), ("w2l", [H, H], f16),
        ("w3h", [H, H], f16), ("w3l", [H, H], f16),
        ("w4h", [H, AOUT], f16), ("w4l", [H, AOUT], f16),
        ("b1", [P, KH], f32), ("b2", [P, KH], f32), ("b3", [P, KH], f32),
        ("b4f", [AOUT, 1], f32),
    ]:
        din[name] = nc.dram_tensor(name, shape, dt_, kind="ExternalInput")
    dout = nc.dram_tensor("v4T", [AOUT, N], f32, kind="ExternalOutput")

    ts = lambda i, sz: slice(i * sz, (i + 1) * sz)

    with tile.TileContext(nc) as tc, ExitStack() as ctx:
        wpool = ctx.enter_context(tc.tile_pool(name="w", bufs=1))
        vpool = ctx.enter_context(tc.tile_pool(name="v", bufs=1))
        spool = ctx.enter_context(tc.tile_pool(name="s", bufs=1))
        upool = ctx.enter_context(tc.tile_pool(name="u", bufs=3))
        tpool = ctx.enter_context(tc.tile_pool(name="t", bufs=3))
        npool = ctx.enter_context(tc.tile_pool(name="n", bufs=2))
        mmps = ctx.enter_context(tc.tile_pool(name="mmps", bufs=3, space="PSUM"))
        zps = ctx.enter_context(tc.tile_pool(name="zps", bufs=1, space="PSUM"))

        def load_km(name, ko, m):
            t_ = wpool.tile([P, ko, m], f16, tag=name)
            nc.sync.dma_start(t_[:], din[name].ap().rearrange("(ko p) m -> p ko m", p=P))
            return t_

        w2h, w2l = load_km("w2h", KH, H), load_km("w2l", KH, H)
        w3h, w3l = load_km("w3h", KH, H), load_km("w3l", KH, H)
        w4h = load_km("w4h", KH, AOUT)

        b1sb = wpool.tile([P, KH], f32, tag="b1")
        nc.sync.dma_start(b1sb[:], din["b1"].ap())
        b2sb = wpool.tile([P, KH], f32, tag="b2")
        nc.sync.dma_start(b2sb[:], din["b2"].ap())
        b3sb = wpool.tile([P, KH], f32, tag="b3")
        nc.sync.dma_start(b3sb[:], din["b3"].ap())
        b4sb = wpool.tile([AOUT, 1], f32, tag="b4f")
        nc.sync.dma_start(b4sb[:], din["b4f"].ap())

        dv1 = vpool.tile([P, KH, N], f32, tag="dv1")
        v1 = vpool.tile([P, KH, N], f32, tag="v1")
        vb2 = vpool.tile([P, KH, N], f32, tag="vb2")
        vb3 = vpool.tile([P, KH, N], f32, tag="vb3")
        s1 = spool.tile([P, KH, N], f16, tag="s1")
        s2 = spool.tile([P, KH, N], f16, tag="s2")
        s3 = spool.tile([P, KH, N], f16, tag="s3")

        nc.gpsimd.memset(v1[:], 0.0)
        nc.gpsimd.memset(vb2[:], 0.0)
        nc.gpsimd.memset(vb3[:], 0.0)
        for c in range(KH):
            nc.scalar.activation(vb2[:, c, :], vb2[:, c, :], IDENT, bias=b2sb[:, ts(c, 1)])
            nc.scalar.activation(vb3[:, c, :], vb3[:, c, :], IDENT, bias=b3sb[:, ts(c, 1)])

        zh = zps.tile([AOUT, N], f32, tag="zh")

        # ---- dv1 = x @ W1.T + b1, in hi/lo pieces (x itself is split too) ----
        def _make_dv1_half(stp, xh, xl):
            def _dv1_half(half, w1h, w1l):
                for cc in range(KH // 2):
                    c = half * (KH // 2) + cc
                    ph = mmps.tile([P, N], f32, tag="ph")
                    pl = mmps.tile([P, N], f32, tag="pl")
                    for k in range(KD):
                        nc.tensor.matmul(ph[:], w1h[:, k, ts(cc, P)], xh[:, k, :],
                                         start=(k == 0), stop=(k == KD - 1))
                    for i, (wt, xt) in enumerate([(w1l, xh), (w1h, xl)]):
                        for k in range(KD):
                            nc.tensor.matmul(pl[:], wt[:, k, ts(cc, P)], xt[:, k, :],
                                             start=(i == 0 and k == 0),
                                             stop=(i == 1 and k == KD - 1))
                    tt = tpool.tile([P, N], f32, tag="t")
                    nc.vector.tensor_scalar(tt[:], pl[:], CLO, None, A.mult)
                    hh = tpool.tile([P, N], f32, tag="t")
                    nc.scalar.activation(hh[:], ph[:], IDENT, bias=b1sb[:, ts(c, 1)])
                    nc.vector.tensor_tensor(dv1[:, c, :], hh[:], tt[:], A.add)
            return _dv1_half

        with tc.tile_pool(name="startup", bufs=1) as stp:
            xh = stp.tile([P, KD, N], f16, tag="xh")
            nc.sync.dma_start(xh[:], din["xh"].ap().rearrange("(ko p) m -> p ko m", p=P))
            xl = stp.tile([P, KD, N], f16, tag="xl")
            nc.sync.dma_start(xl[:], din["xl"].ap().rearrange("(ko p) m -> p ko m", p=P))
            _dv1_half = _make_dv1_half(stp, xh, xl)
            for half in range(2):
                w1h = stp.tile([P, KD, H // 2], f16, tag="w1h")
                nc.sync.dma_start(
                    w1h[:], din["w1h"].ap().rearrange("(ko p) m -> p ko m", p=P)[:, :, ts(half, H // 2)])
                w1l = stp.tile([P, KD, H // 2], f16, tag="w1l")
                nc.sync.dma_start(
                    w1l[:], din["w1l"].ap().rearrange("(ko p) m -> p ko m", p=P)[:, :, ts(half, H // 2)])
                _dv1_half(half, w1h, w1l)

        # ---- the 16-step recurrence ----
        def if_layer(s_in, wh, wl, vb, bsb, s_out, pow2):
            for c in range(KH):
                ph = mmps.tile([P, N], f32, tag="ph")
                pl = mmps.tile([P, N], f32, tag="pl")
                for k in range(KH):
                    nc.tensor.matmul(pl[:], wl[:, k, ts(c, P)], s_in[:, k, :],
                                     start=(k == 0), stop=(k == KH - 1))
                for k in range(KH):
                    nc.tensor.matmul(ph[:], wh[:, k, ts(c, P)], s_in[:, k, :],
                                     start=(k == 0), stop=(k == KH - 1))
                tt = tpool.tile([P, N], f32, tag="t")
                nc.vector.scalar_tensor_tensor(tt[:], pl[:], CLO, vb[:, c, :], A.mult, A.add)
                u = upool.tile([P, N], f32, tag="u")
                nc.vector.tensor_tensor(u[:], ph[:], tt[:], A.add)
                if pow2 is None:
                    nc.gpsimd.tensor_scalar(s_out[:, c, :], u[:], 1.0, None, A.is_ge)
                else:
                    nc.vector.tensor_scalar(s_out[:, c, :], u[:], 1.0, pow2, A.is_ge, A.mult)
                nn = npool.tile([P, N], f16, tag="n")
                nc.gpsimd.tensor_scalar(nn[:], u[:], 1.0, None, A.is_lt)
                t2 = tpool.tile([P, N], f32, tag="t2")
                nc.gpsimd.tensor_tensor(t2[:], u[:], nn[:], A.mult)
                nc.scalar.activation(vb[:, c, :], t2[:], IDENT, bias=bsb[:, ts(c, 1)])

        for t in range(T):
            # layer 1: dv1 is constant; pure elementwise
            for c in range(KH):
                u = upool.tile([P, N], f32, tag="u")
                nc.vector.tensor_tensor(u[:], dv1[:, c, :], v1[:, c, :], A.add)
                nc.gpsimd.tensor_scalar(s1[:, c, :], u[:], 1.0, None, A.is_ge)
                nn = npool.tile([P, N], f16, tag="n")
                nc.gpsimd.tensor_scalar(nn[:], u[:], 1.0, None, A.is_lt)
                nc.vector.tensor_tensor(v1[:, c, :], u[:], nn[:], A.mult)

            if_layer(s1, w2h, w2l, vb2, b2sb, s2, None)
            if_layer(s2, w3h, w3l, vb3, b3sb, s3, float(2.0 ** t))

            for k in range(KH):
                nc.tensor.matmul(zh[:], w4h[:, k, :], s3[:, k, :],
                                 start=(t == 0 and k == 0), stop=(t == T - 1 and k == KH - 1),
                                 skip_group_check=True)

        fout = tpool.tile([AOUT, N], f32, tag="fout")
        nc.scalar.activation(fout[:], zh[:], IDENT, scale=float(2.0 ** -T), bias=b4sb[:])
        nc.sync.dma_start(dout.ap(), fout[:])

    nc.compile()
    return nc


def _hilo(a):
    hi = a.astype(np.float16)
    lo = ((a.astype(np.float32) - hi.astype(np.float32)) * np.float32(2.0 ** 11)).astype(np.float16)
    return hi, lo


def _prep_inputs(x, W1, b1, W2, b2, W3, b3, W4, b4):
    xT = np.ascontiguousarray(x.T.astype(np.float32))          # (D, B)
    xh, xl = _hilo(xT)
    w1h, w1l = _hilo(np.ascontiguousarray(W1.T))               # (D, H)
    w2h, w2l = _hilo(np.ascontiguousarray(W2.T))               # (H, H)
    w3h, w3l = _hilo(np.ascontiguousarray(W3.T))
    w4h, w4l = _hilo(np.ascontiguousarray(W4.T))               # (H, AOUT)
    shared = {
        "w1h": w1h, "w1l": w1l, "w2h": w2h, "w2l": w2l,
        "w3h": w3h, "w3l": w3l, "w4h": w4h, "w4l": w4l,
        "b1": np.ascontiguousarray(b1.reshape(KH, P).T.astype(np.float32)),
        "b2": np.ascontiguousarray(b2.reshape(KH, P).T.astype(np.float32)),
        "b3": np.ascontiguousarray(b3.reshape(KH, P).T.astype(np.float32)),
        "b4f": ((1.0 - 2.0 ** -T) * b4).astype(np.float32).reshape(AOUT, 1),
    }
    in_maps = []
    for i in range(NCORES):
        m = dict(shared)
        m["xh"] = np.ascontiguousarray(xh[:, i * N:(i + 1) * N])
        m["xl"] = np.ascontiguousarray(xl[:, i * N:(i + 1) * N])
        in_maps.append(m)
    return in_maps


def _run(in_maps):
    from concourse.bass_utils import run_bass_kernel_spmd
    if "nc" not in _CACHE:
        _CACHE["nc"] = _build()
    res = run_bass_kernel_spmd(_CACHE["nc"], in_maps, list(range(NCORES)))
    parts = [res.results[i]["v4T"] for i in range(NCORES)]     # each (AOUT, N)
    return np.ascontiguousarray(np.concatenate(parts, axis=1).T).astype(np.float32)


def kernel(x, W1, b1, W2, b2, W3, b3, W4, b4):
    in_maps = _prep_inputs(x, W1, b1, W2, b2, W3, b3, W4, b4)
    return _run(in_maps)



# revision 16
# speedup vs baseline: 1.4579x; 1.4579x over previous
"""Trainium2 Bass kernel for the 4-layer spiking-MLP critic (T=16 IF/LIF recurrence).

Strategy (v3)
- Data-parallel over 8 NeuronCores: batch 4096 -> 512 per core; weights replicated.
- Transposed layout everywhere (features on partitions, batch on the free dim).
- Weight precision via a hi + fp8-lo-chain decomposition. The spike inputs of
  layers 2/3 are *exact* in any dtype, so per output tile one PSUM group sums:
    * 8 fp16 hi matmuls  (W/2 rounded to fp16) x (+-1 fp16 spikes)   [1 cyc/row]
    * 4 DoubleRow fp8 matmuls: e4m3(r*2^16) x (+-2^-16 e5m2 spikes)  [0.5 cyc/row, K=256]
    * 4 DoubleRow fp8 matmuls: e5m2(r2*2^16) x (same moving)
  where r = W/2 - fp16(W/2), r2 = r - 2^-16*Q_a. The 2^16/2^-16 scales cancel
  inside the product, so everything accumulates raw in one fp32 PSUM group.
  Total weight error ~2^-27, matching fp16 hi/lo accuracy at 1.5x fewer cycles.
- Layer-1/2 spikes use a +-1 encoding produced by a single Sign activation:
  W @ s01 == (W/2) @ spm1 + rowsum(W)/2, the constant folded into the bias.
  The +-2^-16 lo spikes are one Identity-activation rescale of the hi spikes.
- Layer 1 (x continuous, time-invariant): dv1 = W1h@xh + W1h@xl + W1ls@xs11 + b1
  with W1ls = fp16(r_w*2^11) and xs11 = fp16(x*2^-11) (scales cancel in-product).
- Membrane u = (psum + b) + v is one fused DVE scalar_tensor_tensor into SBUF;
  spike = Act Sign (+-1); hard reset v' = (u<1)*u on DVE (stt) or Pool (2 ops).
- Layer-4 (non-spiking LIF, tau=2) accumulates in one persistent PSUM bank:
      v4_T = 2^-16 * sum_t 2^t * (s3_t @ W4.T) + (1 - 2^-16) * b4
  with s3 emitted as 0/1 * 2^t in fp16 and W4 in fp16.
- Software-pipelined emission: the PE stream is L2mm(t+1), L3mm(t), L4mm(t), so
  the tensor engine never waits on membrane elementwise work.
"""

import sys

sys.path.insert(0, "/opt/trn_rl_repo")

import numpy as np
import ml_dtypes

P = 128
D, H, AOUT = 512, 1024, 64
N = 512           # batch per core
T = 16
KD, KH = D // P, H // P
NCORES = 8
SLO = 2.0 ** 16   # lo-chain scale

_CACHE = {}


def _build():
    from contextlib import ExitStack
    from concourse import bacc, mybir, tile

    f32 = mybir.dt.float32
    f16 = mybir.dt.float16
    e4 = mybir.dt.float8e4
    e5 = mybir.dt.float8e5
    DR = mybir.MatmulPerfMode.DoubleRow
    A = mybir.AluOpType
    IDENT = mybir.ActivationFunctionType.Identity
    SIGN = mybir.ActivationFunctionType.Sign

    nc = bacc.Bacc("TRN2", target_bir_lowering=False, debug=False)

    din = {}
    for name, shape, dt_ in [
        ("xh", [D, N], f16), ("xl", [D, N], f16), ("xs11", [D, N], f16),
        ("w1h", [D, H], f16), ("w1ls", [D, H], f16),
        ("w2h", [H, H], f16), ("q2a", [H, H], e4), ("q2b", [H, H], e5),
        ("w3h", [H, H], f16), ("q3a", [H, H], e4), ("q3b", [H, H], e5),
        ("w4", [H, AOUT], f16),
        ("b1", [P, KH], f32),
        ("b2", [P, KH], f32),        # b2 + rowsum(W2)/2
        ("b3", [P, KH], f32),        # b3 + rowsum(W3)/2
        ("b4f", [AOUT, 1], f32),
    ]:
        din[name] = nc.dram_tensor(name, shape, dt_, kind="ExternalInput")
    dout = nc.dram_tensor("v4T", [AOUT, N], f32, kind="ExternalOutput")

    cs = lambda c: slice(c * P, (c + 1) * P)

    def ld(tile_, name):
        nc.sync.dma_start(tile_[:], din[name].ap().rearrange("(ko p) m -> p ko m", p=P))

    with tile.TileContext(nc) as tc, ExitStack() as ctx:
        wpool = ctx.enter_context(tc.tile_pool(name="w", bufs=1))
        vpool = ctx.enter_context(tc.tile_pool(name="v", bufs=1))
        spool = ctx.enter_context(tc.tile_pool(name="s", bufs=1))
        u2pool = ctx.enter_context(tc.tile_pool(name="u2", bufs=2))
        u3pool = ctx.enter_context(tc.tile_pool(name="u3", bufs=2))
        nnpool = ctx.enter_context(tc.tile_pool(name="nn", bufs=2))
        p2 = ctx.enter_context(tc.tile_pool(name="p2", bufs=4, space="PSUM"))
        p3 = ctx.enter_context(tc.tile_pool(name="p3", bufs=3, space="PSUM"))
        zps = ctx.enter_context(tc.tile_pool(name="zps", bufs=1, space="PSUM"))

        # ---- biases (tiny DMAs first) ----
        b1sb = wpool.tile([P, KH], f32, tag="b1")
        nc.sync.dma_start(b1sb[:], din["b1"].ap())
        b2sb = wpool.tile([P, KH], f32, tag="b2")
        nc.sync.dma_start(b2sb[:], din["b2"].ap())
        b3sb = wpool.tile([P, KH], f32, tag="b3")
        nc.sync.dma_start(b3sb[:], din["b3"].ap())
        b4sb = wpool.tile([AOUT, 1], f32, tag="b4f")
        nc.sync.dma_start(b4sb[:], din["b4f"].ap())
        negone = wpool.tile([P, 1], f32, tag="negone")
        nc.gpsimd.memset(negone[:], -1.0)

        dv1b = vpool.tile([P, KH, N], f32, tag="dv1b")
        v1 = vpool.tile([P, KH, N], f32, tag="v1")
        v2 = vpool.tile([P, KH, N], f32, tag="v2")
        v3 = vpool.tile([P, KH, N], f32, tag="v3")

        w2h = wpool.tile([P, KH, H], f16, tag="w2h")
        q2a = wpool.tile([P, KH, H], e4, tag="q2a")
        q2b = wpool.tile([P, KH, H], e5, tag="q2b")
        w3h = wpool.tile([P, KH, H], f16, tag="w3h")
        q3a = wpool.tile([P, KH, H], e4, tag="q3a")
        q3b = wpool.tile([P, KH, H], e5, tag="q3b")
        w4sb = wpool.tile([P, KH, AOUT], f16, tag="w4")

        zh = zps.tile([AOUT, N], f32, tag="zh")

        # ---- startup: dv1b = x @ W1.T + b1 (time-invariant; 3 fp16 passes) ----
        with tc.tile_pool(name="startup", bufs=1) as stp:
            xhs = stp.tile([P, KD, N], f16, tag="xh")
            ld(xhs, "xh")
            xls = stp.tile([P, KD, N], f16, tag="xl")
            ld(xls, "xl")
            xss = stp.tile([P, KD, N], f16, tag="xs11")
            ld(xss, "xs11")
            w1hs = stp.tile([P, KD, H], f16, tag="w1h")
            ld(w1hs, "w1h")
            w1ls = stp.tile([P, KD, H], f16, tag="w1ls")
            ld(w1ls, "w1ls")
            # big weights queue behind the L1 operands on the DMA engine
            ld(w2h, "w2h"); ld(q2a, "q2a"); ld(q2b, "q2b")
            ld(w3h, "w3h"); ld(q3a, "q3a"); ld(q3b, "q3b")
            ld(w4sb, "w4")

            for c in range(KH):
                ps = p2.tile([P, N], f32, tag="p2")
                for k in range(KD):
                    nc.tensor.matmul(ps[:], w1hs[:, k, cs(c)], xhs[:, k, :],
                                     start=(k == 0), stop=False)
                for k in range(KD):
                    nc.tensor.matmul(ps[:], w1hs[:, k, cs(c)], xls[:, k, :],
                                     start=False, stop=False)
                for k in range(KD):
                    nc.tensor.matmul(ps[:], w1ls[:, k, cs(c)], xss[:, k, :],
                                     start=False, stop=(k == KD - 1))
                nc.scalar.activation(dv1b[:, c, :], ps[:], IDENT, bias=b1sb[:, c:c + 1])

        # ---- t=0 layer-1 membranes (v1 starts at 0, so u = dv1b) ----
        s1t = spool.tile([P, KH, N], f16, tag="s1")
        nc.scalar.activation(s1t[:], dv1b[:], SIGN, bias=negone[:])
        s1lo = spool.tile([P, KH, N], e5, tag="s1lo")
        nc.scalar.activation(s1lo[:], s1t[:], IDENT, scale=float(1.0 / SLO))
        nc.vector.scalar_tensor_tensor(v1[:], dv1b[:], 1.0, dv1b[:], A.is_lt, A.mult)

        def emit_mm(pool_, wh, qa, qb, s_hi, s_lo):
            tiles = []
            for c in range(KH):
                ps = pool_.tile([P, N], f32, tag=pool_.name)
                for k in range(KH):
                    nc.tensor.matmul(ps[:], wh[:, k, cs(c)], s_hi[:, k, :],
                                     start=(k == 0), stop=False)
                for kk in range(KH // 2):
                    nc.tensor.matmul(ps[:], qa[:, 2 * kk:2 * kk + 2, cs(c)],
                                     s_lo[:, 2 * kk:2 * kk + 2, :],
                                     start=False, stop=False, perf_mode=DR)
                for kk in range(KH // 2):
                    nc.tensor.matmul(ps[:], qb[:, 2 * kk:2 * kk + 2, cs(c)],
                                     s_lo[:, 2 * kk:2 * kk + 2, :],
                                     start=False, stop=(kk == KH // 2 - 1), perf_mode=DR)
                tiles.append(ps)
            return tiles

        ps2 = emit_mm(p2, w2h, q2a, q2b, s1t, s1lo)

        # ---- the 16-step recurrence, software pipelined ----
        for t in range(T):
            last = (t == T - 1)

            # L2 membranes for step t: u = (psum + b2) + v2 into SBUF (one fused
            # DVE op), spike via Act Sign, lo-spike rescale on Act, reset on DVE
            s2t = spool.tile([P, KH, N], f16, tag="s2")
            s2lo = spool.tile([P, KH, N], e5, tag="s2lo")
            for c in range(KH):
                ps = ps2[c]
                u2 = u2pool.tile([P, N], f32, tag="u2")
                if t == 0:
                    nc.vector.tensor_scalar(u2[:], ps[:], b2sb[:, c:c + 1], None, A.add)
                else:
                    nc.vector.scalar_tensor_tensor(u2[:], ps[:], b2sb[:, c:c + 1],
                                                   v2[:, c, :], A.add, A.add)
                nc.scalar.activation(s2t[:, c, :], u2[:], SIGN, bias=negone[:])
                if not last:
                    nc.vector.scalar_tensor_tensor(v2[:, c, :], u2[:], 1.0, u2[:],
                                                   A.is_lt, A.mult)
            nc.scalar.activation(s2lo[:], s2t[:], IDENT, scale=float(1.0 / SLO))

            # L1 membranes for step t+1 -> s1(t+1); then L2 matmuls for t+1
            # (in-place: v1 += dv1b; spike; hard reset)
            if not last:
                nc.vector.tensor_tensor(v1[:], dv1b[:], v1[:], A.add)
                s1t = spool.tile([P, KH, N], f16, tag="s1")
                nc.scalar.activation(s1t[:], v1[:], SIGN, bias=negone[:])
                s1lo = spool.tile([P, KH, N], e5, tag="s1lo")
                nc.scalar.activation(s1lo[:], s1t[:], IDENT, scale=float(1.0 / SLO))
                if t + 2 < T:
                    nc.vector.scalar_tensor_tensor(v1[:], v1[:], 1.0, v1[:],
                                                   A.is_lt, A.mult)
                ps2_next = emit_mm(p2, w2h, q2a, q2b, s1t, s1lo)

            # L3 matmuls for step t
            ps3 = emit_mm(p3, w3h, q3a, q3b, s2t, s2lo)

            # L3 membranes for step t -> s3t (0/1 scaled by 2^t), update v3 on
            # Pool (via SBUF u3; GPSIMD cannot access PSUM)
            s3t = spool.tile([P, KH, N], f16, tag="s3")
            pw = float(2.0 ** t)
            for c in range(KH):
                ps = ps3[c]
                u3 = u3pool.tile([P, N], f32, tag="u3")
                if t == 0:
                    nc.vector.tensor_scalar(u3[:], ps[:], b3sb[:, c:c + 1], None, A.add)
                else:
                    nc.vector.scalar_tensor_tensor(u3[:], ps[:], b3sb[:, c:c + 1],
                                                   v3[:, c, :], A.add, A.add)
                nc.vector.tensor_scalar(s3t[:, c, :], u3[:], 1.0, pw, A.is_ge, A.mult)
                if not last:
                    nn = nnpool.tile([P, N], f16, tag="nn")
                    nc.gpsimd.tensor_scalar(nn[:], u3[:], 1.0, None, A.is_lt)
                    nc.gpsimd.tensor_tensor(v3[:, c, :], u3[:], nn[:], A.mult)

            # L4: accumulate 2^t-weighted spikes into the persistent PSUM bank
            for k in range(KH):
                nc.tensor.matmul(zh[:], w4sb[:, k, :], s3t[:, k, :],
                                 start=(t == 0 and k == 0), stop=(last and k == KH - 1),
                                 skip_group_check=True)

            if not last:
                ps2 = ps2_next

        fout = vpool.tile([AOUT, N], f32, tag="fout")
        nc.scalar.activation(fout[:], zh[:], IDENT, scale=float(2.0 ** -T), bias=b4sb[:])
        nc.sync.dma_start(dout.ap(), fout[:])

    nc.compile()
    return nc


def _f16(a):
    return a.astype(np.float16).astype(np.float32)


def _lo_chain(wT):
    """wT (f32, transposed, already halved) -> (hi f16, Qa e4m3, Qb e5m2)."""
    hi = wT.astype(np.float16)
    r = (wT.astype(np.float64) - hi.astype(np.float64)).astype(np.float32)
    qa = (r * SLO).astype(ml_dtypes.float8_e4m3)
    r2 = (r.astype(np.float64) - qa.astype(np.float64) / SLO).astype(np.float32)
    qb = (r2 * SLO).astype(ml_dtypes.float8_e5m2)
    return hi, qa, qb


def _prep_inputs(x, W1, b1, W2, b2, W3, b3, W4, b4):
    f32 = np.float32
    xT = np.ascontiguousarray(x.T.astype(f32))                  # (D, B)
    xh = xT.astype(np.float16)
    xl = (xT - xh.astype(f32)).astype(np.float16)
    xs11 = (xT * np.float32(2.0 ** -11)).astype(np.float16)

    w1T = np.ascontiguousarray(W1.T.astype(f32))                # (D, H)
    w1h = w1T.astype(np.float16)
    w1ls = ((w1T.astype(np.float64) - w1h.astype(np.float64)) * 2.0 ** 11
            ).astype(np.float16)

    w2h, q2a, q2b = _lo_chain(np.ascontiguousarray(W2.T.astype(f32)) / 2.0)
    w3h, q3a, q3b = _lo_chain(np.ascontiguousarray(W3.T.astype(f32)) / 2.0)

    b2e = (b2.astype(np.float64) + W2.astype(np.float64).sum(axis=1) / 2.0).astype(f32)
    b3e = (b3.astype(np.float64) + W3.astype(np.float64).sum(axis=1) / 2.0).astype(f32)
    shared = {
        "w1h": np.ascontiguousarray(w1h), "w1ls": np.ascontiguousarray(w1ls),
        "w2h": np.ascontiguousarray(w2h), "q2a": np.ascontiguousarray(q2a),
        "q2b": np.ascontiguousarray(q2b),
        "w3h": np.ascontiguousarray(w3h), "q3a": np.ascontiguousarray(q3a),
        "q3b": np.ascontiguousarray(q3b),
        "w4": np.ascontiguousarray(W4.T.astype(np.float16)),
        "b1": np.ascontiguousarray(b1.reshape(KH, P).T.astype(f32)),
        "b2": np.ascontiguousarray(b2e.reshape(KH, P).T),
        "b3": np.ascontiguousarray(b3e.reshape(KH, P).T),
        "b4f": ((1.0 - 2.0 ** -T) * b4).astype(f32).reshape(AOUT, 1),
    }
    in_maps = []
    for i in range(NCORES):
        m = dict(shared)
        sl = slice(i * N, (i + 1) * N)
        m["xh"] = np.ascontiguousarray(xh[:, sl])
        m["xl"] = np.ascontiguousarray(xl[:, sl])
        m["xs11"] = np.ascontiguousarray(xs11[:, sl])
        in_maps.append(m)
    return in_maps


def _run(in_maps):
    from concourse.bass_utils import run_bass_kernel_spmd
    if "nc" not in _CACHE:
        _CACHE["nc"] = _build()
    res = run_bass_kernel_spmd(_CACHE["nc"], in_maps, list(range(NCORES)))
    parts = [res.results[i]["v4T"] for i in range(NCORES)]      # each (AOUT, N)
    return np.ascontiguousarray(np.concatenate(parts, axis=1).T).astype(np.float32)


def kernel(x, W1, b1, W2, b2, W3, b3, W4, b4):
    in_maps = _prep_inputs(x, W1, b1, W2, b2, W3, b3, W4, b4)
    return _run(in_maps)
